# revision 1
# baseline (speedup 1.0000x reference)
"""Expert-parallel MoE GLU MLP kernel for Trainium2.

8 experts -> 8 NeuronCores, one expert per core (no collectives needed).
Per core:  x (C,H) @ w_gate_up (H,2I) -> GLU -> (C,I) @ w_down (I,H) -> (C,H)

Strategy (per core), all matmuls bf16 (fp32 accumulate in PSUM):
  - Host passes x pre-transposed (H,C) per expert; streamed in c-half pieces
    and cast f32->bf16 on the Scalar engine (keeps DVE free for weight
    casts), so the first GEMM1 chain starts after ~4 MB.
  - GEMM1: stationary = w_gate_up column slices [128h x 128f] (f32 load +
    DVE cast), moving = xT -> psum (f, c). GLU = silu(gate) [ACT] * up [DVE]
    written straight into a bf16 SBUF-resident act tile (I, C) - no DRAM
    round-trip for activations.
  - GEMM2: stationary = act tiles [128i x 128c], moving = w_down h-slabs
    (f32 load + DVE cast, split in i-quarters so chains start early) ->
    psum accumulates the full 32-tile I chain -> copy -> out (C,H).
    Slab 0 is prefetched during phase 1 (zero phase-boundary bubble).
  - Single PSUM pool for both phases (no pool-release barrier), and all
    matmuls one dtype (avoids the fp32r<->bf16 FWL interleaving hazard
    that crashes the PE).
"""
import numpy as np

E, C, H, I = 8, 1024, 2048, 4096
P = 128
HT, IT, CT = H // P, I // P, C // P  # 16, 32, 8

_CACHE = {}


def _build():
    import concourse.bacc as bacc
    import concourse.mybir as mybir
    import concourse.tile as tile

    f32 = mybir.dt.float32
    bf16 = mybir.dt.bfloat16
    AF = mybir.ActivationFunctionType

    nc = bacc.Bacc("TRN2", target_bir_lowering=False, debug=False)
    xT = nc.declare_dram_parameter("xT", [H, C], f32, isOutput=False).ap()
    wgu = nc.declare_dram_parameter("wgu", [H, 2 * I], f32, isOutput=False).ap()
    wdn = nc.declare_dram_parameter("wdn", [I, H], f32, isOutput=False).ap()
    out = nc.declare_dram_parameter("out", [C, H], f32, isOutput=True).ap()

    xT_v = xT.rearrange("(ht p) c -> p ht c", p=P)    # [128, 16, 1024]
    wgu_v = wgu.rearrange("(ht p) f -> p ht f", p=P)  # [128, 16, 8192]
    wdn_v = wdn.rearrange("(it p) h -> p it h", p=P)  # [128, 32, 2048]
    out_v = out.rearrange("(ct p) h -> p ct h", p=P)  # [128, 8, 2048]

    NHC = 8               # w_down h-slabs
    HW_ = H // NHC        # 256 cols per slab
    NSUB = 4              # i-quarters per slab load/cast
    ISUB = IT // NSUB     # 8 i-tiles per quarter

    with tile.TileContext(nc) as tc:
        with (
            tc.tile_pool(name="acts_pool", bufs=1) as actsp,
            tc.tile_pool(name="pp", bufs=1, space="PSUM") as pp,
            tc.tile_pool(name="sbs", bufs=3) as sbs,
        ):
            # acts[p, it, c] = act row (it*128+p), col c  (bf16, 8 MB)
            acts = actsp.tile([P, IT, C], bf16)

            # slab 0 of w_down is prefetched during phase 1 from this small
            # pool so the phase boundary has zero weight-load bubble
            w2pre_cm = tc.tile_pool(name="w2pre", bufs=1)
            w2pre = w2pre_cm.__enter__()
            wsl0 = w2pre.tile([P, IT, H // 8], bf16, name="wsl0")

            # ---- Phase 1: gate_up GEMM + GLU -> acts ---------------------
            with (
                tc.tile_pool(name="xt_pool", bufs=1) as xtp,
                tc.tile_pool(name="w1", bufs=2) as w1,
            ):
                xt = xtp.tile([P, HT, C], bf16)  # xt[p, ht, c] = x[c, ht*128+p]

                def load_w1(i):
                    wgf = w1.tile([P, HT, P], f32, tag="wgf", name=f"wgf{i}")
                    nc.sync.dma_start(wgf, wgu_v[:, :, i * P:(i + 1) * P])
                    wg = w1.tile([P, HT, P], bf16, tag="wg", name=f"wg{i}")
                    nc.vector.tensor_copy(wg, wgf)
                    wuf = w1.tile([P, HT, P], f32, tag="wuf", name=f"wuf{i}")
                    nc.sync.dma_start(
                        wuf, wgu_v[:, :, I + i * P:I + (i + 1) * P])
                    wu = w1.tile([P, HT, P], bf16, tag="wu", name=f"wu{i}")
                    nc.vector.tensor_copy(wu, wuf)
                    return wg, wu

                # first weight pair issues ahead of the xT stream
                w_next = load_w1(0)
                # xT: cc=0 halves first so chain (i=0, cc=0) unblocks early
                for cc in range(2):
                    for ht in range(HT):
                        cs = slice(cc * 512, (cc + 1) * 512)
                        xs = w1.tile([P, 512], f32, tag="xs",
                                     name=f"xs{cc}_{ht}", bufs=3)
                        nc.sync.dma_start(xs, xT_v[:, ht, cs])
                        # cast on ACT: keeps DVE free for the weight casts
                        nc.scalar.activation(xt[:, ht, cs], xs, AF.Identity)

                for i in range(IT):
                    wg, wu = w_next
                    if i + 1 < IT:
                        w_next = load_w1(i + 1)
                    if 8 <= i < 12:
                        # slab-0 quarter loads, spread mid-phase-1
                        q = i - 8
                        qs = slice(q * (IT // 4), (q + 1) * (IT // 4))
                        w0f = w1.tile([P, IT // 4, H // 8], f32, tag="w0f",
                                      name=f"w0f{q}", bufs=2)
                        nc.sync.dma_start(w0f, wdn_v[:, qs, 0:H // 8])
                        nc.vector.tensor_copy(wsl0[:, qs, :], w0f)
                    for cc in range(2):
                        cs = slice(cc * 512, (cc + 1) * 512)
                        pg = pp.tile([P, 512], f32, tag="pg", bufs=2)
                        pu = pp.tile([P, 512], f32, tag="pu", bufs=2)
                        for ht in range(HT):
                            nc.tensor.matmul(
                                pg, wg[:, ht, :], xt[:, ht, cs],
                                start=(ht == 0), stop=(ht == HT - 1))
                        for ht in range(HT):
                            nc.tensor.matmul(
                                pu, wu[:, ht, :], xt[:, ht, cs],
                                start=(ht == 0), stop=(ht == HT - 1))
                        sil = sbs.tile([P, 512], f32, tag="sil")
                        nc.scalar.activation(sil, pg, AF.Silu)
                        nc.vector.tensor_mul(acts[:, i, cs], sil, pu)

            # ---- Phase 2: down GEMM (bf16), full-I psum chains -----------
            with tc.tile_pool(name="w2", bufs=2) as w2:
                for hc in range(NHC):
                    hs = slice(hc * HW_, (hc + 1) * HW_)
                    if hc == 0:
                        wsl = wsl0
                    else:
                        wsf = w2.tile([P, IT, HW_], f32, tag="wsf")
                        wsl = w2.tile([P, IT, HW_], bf16, tag="wsl")
                        for q in range(NSUB):
                            qs = slice(q * ISUB, (q + 1) * ISUB)
                            nc.sync.dma_start(wsf[:, qs, :], wdn_v[:, qs, hs])
                            nc.vector.tensor_copy(wsl[:, qs, :], wsf[:, qs, :])
                    for ct in range(CT):
                        ps = pp.tile([P, HW_], f32, tag="ps", bufs=4)
                        for i in range(IT):
                            nc.tensor.matmul(
                                ps,
                                acts[:, i, ct * P:(ct + 1) * P],
                                wsl[:, i, :],
                                start=(i == 0), stop=(i == IT - 1))
                        osb = sbs.tile([P, HW_], f32, tag="osb")
                        nc.vector.tensor_copy(osb, ps)
                        nc.sync.dma_start(out_v[:, ct, hs], osb)
            w2pre_cm.__exit__(None, None, None)

    nc.compile()
    return nc


def _get_nc():
    if "nc" not in _CACHE:
        _CACHE["nc"] = _build()
    return _CACHE["nc"]


def _run(hidden_states, w_gate_up, w_down, trace=False):
    from concourse.bass_utils import run_bass_kernel_spmd

    nc = _get_nc()
    hs = np.asarray(hidden_states, dtype=np.float32)
    wg = np.ascontiguousarray(np.asarray(w_gate_up, dtype=np.float32))
    wd = np.ascontiguousarray(np.asarray(w_down, dtype=np.float32))
    in_maps = [
        {
            "xT": np.ascontiguousarray(hs[e].T),
            "wgu": wg[e],
            "wdn": wd[e],
        }
        for e in range(E)
    ]
    res = run_bass_kernel_spmd(nc, in_maps, list(range(E)), trace=trace)
    output = np.stack([res.results[e]["out"] for e in range(E)], axis=0)
    return output, res


def kernel(hidden_states, w_gate_up, w_down):
    output, _ = _run(hidden_states, w_gate_up, w_down, trace=False)
    return output



# revision 2
# speedup vs baseline: 1.0549x; 1.0549x over previous
"""Expert-parallel MoE GLU MLP kernel for Trainium2.

8 experts -> 8 NeuronCores, one expert per core (no collectives needed).
Per core:  x (C,H) @ w_gate_up (H,2I) -> GLU -> (C,I) @ w_down (I,H) -> (C,H)

Strategy (per core), all matmuls bf16 (fp32 accumulate in PSUM):
  - Host pre-casts x (transposed to (H,C)), w_gate_up and w_down to bf16,
    so the device does zero dtype conversion and weight DMA bytes are
    halved vs f32.  Total DMA: 32+16+4 MB in, 8 MB out = 60 MB (~170 us)
    under ~660 us of PE work -> fully compute-bound.
  - PE warm-up: a burst of junk matmuls on a zeroed tile fills the HAM
    activity window during the initial x/w DMAs so real chains start at
    2.4 GHz instead of 1.2 GHz.
  - GEMM1: stationary = w_gate_up column tiles loaded in 256-col pairs
    (512 B DMA chunks = SDMA line rate), moving = xT (4 chunked DMAs so
    the first chain unblocks after ~1 MB).  GLU = silu(gate) [ACT] *
    up [DVE] written into a bf16 SBUF-resident act tile (I, C).
  - GEMM2: stationary = act tiles [128i x 128c], moving = w_down h-slabs
    of 512 cols (N=512 matmuls hide LDWEIGHTS fully) -> psum accumulates
    the full 32-tile I chain -> copy -> out (C,H).  Slab 0 is prefetched
    during phase 1; slab hc+1 is prefetched at the start of slab hc's
    chains (before hc's out-DMAs are queued, avoiding head-of-line
    blocking on the sync DMA queue).  Out-DMAs ride the scalar queue.
  - Single PSUM pool for both phases, all matmuls one dtype (avoids the
    dtype-interleave FWL hazard that crashes the PE).
"""
import numpy as np
import ml_dtypes

E, C, H, I = 8, 1024, 2048, 4096
P = 128
HT, IT, CT = H // P, I // P, C // P  # 16, 32, 8
NP1 = IT // 2         # 16 gate/up weight pair-blocks (256 cols each)
NHC = 4               # w_down h-slabs
HW_ = H // NHC        # 512 cols per slab
WARM = 24             # junk matmuls to warm the PE clock gate

_CACHE = {}


def _build():
    import concourse.bacc as bacc
    import concourse.mybir as mybir
    import concourse.tile as tile

    f32 = mybir.dt.float32
    bf16 = mybir.dt.bfloat16
    AF = mybir.ActivationFunctionType

    nc = bacc.Bacc("TRN2", target_bir_lowering=False, debug=False)
    xT = nc.declare_dram_parameter("xT", [H, C], bf16, isOutput=False).ap()
    wgu = nc.declare_dram_parameter("wgu", [H, 2 * I], bf16, isOutput=False).ap()
    wdn = nc.declare_dram_parameter("wdn", [I, H], bf16, isOutput=False).ap()
    out = nc.declare_dram_parameter("out", [C, H], f32, isOutput=True).ap()

    xT_v = xT.rearrange("(ht p) c -> p ht c", p=P)    # [128, 16, 1024]
    wgu_v = wgu.rearrange("(ht p) f -> p ht f", p=P)  # [128, 16, 8192]
    wdn_v = wdn.rearrange("(it p) h -> p it h", p=P)  # [128, 32, 2048]
    out_v = out.rearrange("(ct p) h -> p ct h", p=P)  # [128, 8, 2048]

    with tile.TileContext(nc) as tc:
        with (
            tc.tile_pool(name="acts_pool", bufs=1) as actsp,
            tc.tile_pool(name="pp", bufs=1, space="PSUM") as pp,
            tc.tile_pool(name="sbs", bufs=3) as sbs,
            tc.tile_pool(name="w2pre", bufs=1) as w2pre,
        ):
            # acts[p, it, c] = act row (it*128+p), col c  (bf16, 8 MB)
            acts = actsp.tile([P, IT, C], bf16)
            # slab 0 of w_down, prefetched during phase 1
            wsl0 = w2pre.tile([P, IT, HW_], bf16, name="wsl0")

            # ---- Phase 1: gate_up GEMM + GLU -> acts ---------------------
            with (
                tc.tile_pool(name="xt_pool", bufs=1) as xtp,
                tc.tile_pool(name="w1", bufs=2) as w1,
            ):
                # PE warm-up on a zeroed tile (no data deps -> runs during
                # the initial DMAs; ~2.5 us of junk matmuls)
                wtile = sbs.tile([P, P], bf16, tag="warm", bufs=1)
                nc.vector.memset(wtile, 0.0)
                pw = pp.tile([P, P], f32, tag="wm", bufs=1)
                for w in range(WARM):
                    nc.tensor.matmul(pw, wtile, wtile, start=True, stop=True)

                xt = xtp.tile([P, HT, C], bf16)  # xt[p, ht, c] = x[c, ht*128+p]
                # x: 4 chunked DMAs; (cc=0, ht 0-7) first so chain 0 can
                # start after ~1 MB
                for cc in range(2):
                    cs = slice(cc * 512, (cc + 1) * 512)
                    for hh in range(2):
                        hs = slice(hh * 8, (hh + 1) * 8)
                        nc.sync.dma_start(xt[:, hs, cs], xT_v[:, hs, cs])

                def load_pair(j):
                    fs = slice(2 * j * P, (2 * j + 2) * P)
                    us = slice(I + 2 * j * P, I + (2 * j + 2) * P)
                    wg = w1.tile([P, HT, 2 * P], bf16, tag="wg", name=f"wg{j}")
                    nc.sync.dma_start(wg, wgu_v[:, :, fs])
                    wu = w1.tile([P, HT, 2 * P], bf16, tag="wu", name=f"wu{j}")
                    nc.sync.dma_start(wu, wgu_v[:, :, us])
                    return wg, wu

                w_next = load_pair(0)
                for j in range(NP1):
                    wgp, wup = w_next
                    if j + 1 < NP1:
                        w_next = load_pair(j + 1)
                    if 4 <= j < 8:
                        # slab-0 quarter loads, spread mid-phase-1
                        q = j - 4
                        qs = slice(q * (IT // 4), (q + 1) * (IT // 4))
                        nc.sync.dma_start(wsl0[:, qs, :], wdn_v[:, qs, 0:HW_])
                    for k in range(2):
                        i = 2 * j + k
                        ks = slice(k * P, (k + 1) * P)
                        for cc in range(2):
                            cs = slice(cc * 512, (cc + 1) * 512)
                            pg = pp.tile([P, 512], f32, tag="pg", bufs=2)
                            pu = pp.tile([P, 512], f32, tag="pu", bufs=2)
                            for ht in range(HT):
                                nc.tensor.matmul(
                                    pg, wgp[:, ht, ks], xt[:, ht, cs],
                                    start=(ht == 0), stop=(ht == HT - 1))
                            for ht in range(HT):
                                nc.tensor.matmul(
                                    pu, wup[:, ht, ks], xt[:, ht, cs],
                                    start=(ht == 0), stop=(ht == HT - 1))
                            sil = sbs.tile([P, 512], f32, tag="sil")
                            nc.scalar.activation(sil, pg, AF.Silu)
                            nc.vector.tensor_mul(acts[:, i, cs], sil, pu)

            # ---- Phase 2: down GEMM (bf16), full-I psum chains -----------
            with tc.tile_pool(name="w2", bufs=2) as w2:

                def load_slab(hc):
                    hs = slice(hc * HW_, (hc + 1) * HW_)
                    wsl = w2.tile([P, IT, HW_], bf16, tag="wsl", name=f"ws{hc}")
                    for q in range(4):
                        qs = slice(q * (IT // 4), (q + 1) * (IT // 4))
                        nc.sync.dma_start(wsl[:, qs, :], wdn_v[:, qs, hs])
                    return wsl

                wsl_cur = wsl0
                wsl_next = load_slab(1)
                for hc in range(NHC):
                    hs = slice(hc * HW_, (hc + 1) * HW_)
                    for ct in range(CT):
                        ps = pp.tile([P, HW_], f32, tag="ps", bufs=2)
                        for i in range(IT):
                            nc.tensor.matmul(
                                ps,
                                acts[:, i, ct * P:(ct + 1) * P],
                                wsl_cur[:, i, :],
                                start=(i == 0), stop=(i == IT - 1))
                        osb = sbs.tile([P, HW_], f32, tag="osb")
                        nc.vector.tensor_copy(osb, ps)
                        # out-DMAs ride the scalar HWDGE queue so slab
                        # prefetches on sync are never stuck behind them
                        nc.scalar.dma_start(out_v[:, ct, hs], osb)
                    if hc + 1 < NHC:
                        wsl_cur = wsl_next
                        if hc + 2 < NHC:
                            wsl_next = load_slab(hc + 2)

    nc.compile()
    return nc


def _get_nc():
    if "nc" not in _CACHE:
        _CACHE["nc"] = _build()
    return _CACHE["nc"]


def _bf16(a):
    return np.ascontiguousarray(np.asarray(a, dtype=ml_dtypes.bfloat16))


def _run(hidden_states, w_gate_up, w_down, trace=False):
    from concourse.bass_utils import run_bass_kernel_spmd

    nc = _get_nc()
    hs = np.asarray(hidden_states, dtype=np.float32)
    in_maps = [
        {
            "xT": _bf16(hs[e].T),
            "wgu": _bf16(w_gate_up[e]),
            "wdn": _bf16(w_down[e]),
        }
        for e in range(E)
    ]
    res = run_bass_kernel_spmd(nc, in_maps, list(range(E)), trace=trace)
    output = np.stack([res.results[e]["out"] for e in range(E)], axis=0)
    return output, res


def kernel(hidden_states, w_gate_up, w_down):
    output, _ = _run(hidden_states, w_gate_up, w_down, trace=False)
    return output


# revision 3
# speedup vs baseline: 1.0655x; 1.0100x over previous
"""Expert-parallel MoE GLU MLP kernel for Trainium2.

8 experts -> 8 NeuronCores, one expert per core (no collectives needed).
Per core:  x (C,H) @ w_gate_up (H,2I) -> GLU -> (C,I) @ w_down (I,H) -> (C,H)

Strategy (per core), all matmuls bf16 (fp32 accumulate in PSUM):
  - Host pre-casts x (transposed to (H,C)), w_gate_up and w_down to bf16,
    so the device does zero dtype conversion and weight DMA bytes are
    halved vs f32.  Total DMA: 32+16+4 MB in, 8 MB out = 60 MB (~170 us)
    under ~660 us of PE work -> fully compute-bound.
  - PE warm-up: a burst of junk matmuls on a zeroed tile fills the HAM
    activity window during the initial x/w DMAs so real chains start at
    2.4 GHz instead of 1.2 GHz.
  - GEMM1: stationary = w_gate_up column tiles loaded in 256-col pairs
    (512 B DMA chunks = SDMA line rate), moving = xT (4 chunked DMAs so
    the first chain unblocks after ~1 MB).  GLU = silu(gate) [ACT] *
    up [DVE] written into a bf16 SBUF-resident act tile (I, C).
  - GEMM2: stationary = act tiles [128i x 128c], moving = w_down h-slabs
    of 512 cols (N=512 matmuls hide LDWEIGHTS fully) -> psum accumulates
    the full 32-tile I chain -> copy -> out (C,H).  Slab 0 is prefetched
    during phase 1; slab hc+1 is prefetched at the start of slab hc's
    chains (before hc's out-DMAs are queued, avoiding head-of-line
    blocking on the sync DMA queue).  Out-DMAs ride the scalar queue.
  - Single PSUM pool for both phases, all matmuls one dtype (avoids the
    dtype-interleave FWL hazard that crashes the PE).
"""
import numpy as np
import ml_dtypes

E, C, H, I = 8, 1024, 2048, 4096
P = 128
HT, IT, CT = H // P, I // P, C // P  # 16, 32, 8
NP1 = IT // 2         # 16 gate/up weight pair-blocks (256 cols each)
NHC = 4               # w_down h-slabs
HW_ = H // NHC        # 512 cols per slab
WARM = 24             # junk matmuls to warm the PE clock gate

_CACHE = {}


def _build():
    import concourse.bacc as bacc
    import concourse.mybir as mybir
    import concourse.tile as tile

    f32 = mybir.dt.float32
    bf16 = mybir.dt.bfloat16
    AF = mybir.ActivationFunctionType

    nc = bacc.Bacc("TRN2", target_bir_lowering=False, debug=False)
    xT = nc.declare_dram_parameter("xT", [H, C], bf16, isOutput=False).ap()
    wgu = nc.declare_dram_parameter("wgu", [H, 2 * I], bf16, isOutput=False).ap()
    wdn = nc.declare_dram_parameter("wdn", [I, H], bf16, isOutput=False).ap()
    out = nc.declare_dram_parameter("out", [C, H], f32, isOutput=True).ap()

    xT_v = xT.rearrange("(ht p) c -> p ht c", p=P)    # [128, 16, 1024]
    wgu_v = wgu.rearrange("(ht p) f -> p ht f", p=P)  # [128, 16, 8192]
    wdn_v = wdn.rearrange("(it p) h -> p it h", p=P)  # [128, 32, 2048]
    out_v = out.rearrange("(ct p) h -> p ct h", p=P)  # [128, 8, 2048]

    with tile.TileContext(nc) as tc:
        with (
            tc.tile_pool(name="acts_pool", bufs=1) as actsp,
            tc.tile_pool(name="pp", bufs=1, space="PSUM") as pp,
            tc.tile_pool(name="sbs", bufs=3) as sbs,
            tc.tile_pool(name="w2pre", bufs=1) as w2pre,
        ):
            # acts[p, it, c] = act row (it*128+p), col c  (bf16, 8 MB)
            acts = actsp.tile([P, IT, C], bf16)
            # slab 0 of w_down, prefetched during phase 1
            wsl0 = w2pre.tile([P, IT, HW_], bf16, name="wsl0")

            # ---- Phase 1: gate_up GEMM + GLU -> acts ---------------------
            with (
                tc.tile_pool(name="xt_pool", bufs=1) as xtp,
                tc.tile_pool(name="w1", bufs=2) as w1,
            ):
                # PE warm-up on a zeroed tile (no data deps -> runs during
                # the initial DMAs; ~2.5 us of junk matmuls)
                wtile = sbs.tile([P, P], bf16, tag="warm", bufs=1)
                nc.vector.memset(wtile, 0.0)
                pw = pp.tile([P, P], f32, tag="wm", bufs=1)
                for w in range(WARM):
                    nc.tensor.matmul(pw, wtile, wtile, start=True, stop=True)

                xt = xtp.tile([P, HT, C], bf16)  # xt[p, ht, c] = x[c, ht*128+p]

                def load_pair(j, split=False):
                    fs = slice(2 * j * P, (2 * j + 2) * P)
                    us = slice(I + 2 * j * P, I + (2 * j + 2) * P)
                    wg = w1.tile([P, HT, 2 * P], bf16, tag="wg", name=f"wg{j}")
                    wu = w1.tile([P, HT, 2 * P], bf16, tag="wu", name=f"wu{j}")
                    if split:
                        # first pair: interleave with the x chunks in the
                        # order the first chains consume them, so chain 0
                        # unblocks after ~1.5 MB instead of 5 MB
                        h0, h1 = slice(0, 8), slice(8, 16)
                        nc.sync.dma_start(wg[:, h0, :], wgu_v[:, h0, fs])
                        nc.sync.dma_start(xt[:, h0, 0:512], xT_v[:, h0, 0:512])
                        nc.sync.dma_start(wg[:, h1, :], wgu_v[:, h1, fs])
                        nc.sync.dma_start(xt[:, h1, 0:512], xT_v[:, h1, 0:512])
                        nc.sync.dma_start(wu[:, h0, :], wgu_v[:, h0, us])
                        nc.sync.dma_start(wu[:, h1, :], wgu_v[:, h1, us])
                        nc.sync.dma_start(xt[:, h0, 512:1024],
                                          xT_v[:, h0, 512:1024])
                        nc.sync.dma_start(xt[:, h1, 512:1024],
                                          xT_v[:, h1, 512:1024])
                    else:
                        nc.sync.dma_start(wg, wgu_v[:, :, fs])
                        nc.sync.dma_start(wu, wgu_v[:, :, us])
                    return wg, wu

                w_next = load_pair(0, split=True)
                for j in range(NP1):
                    wgp, wup = w_next
                    if j + 1 < NP1:
                        w_next = load_pair(j + 1)
                    if 4 <= j < 8:
                        # slab-0 quarter loads, spread mid-phase-1
                        q = j - 4
                        qs = slice(q * (IT // 4), (q + 1) * (IT // 4))
                        nc.sync.dma_start(wsl0[:, qs, :], wdn_v[:, qs, 0:HW_])
                    for k in range(2):
                        i = 2 * j + k
                        ks = slice(k * P, (k + 1) * P)
                        for cc in range(2):
                            cs = slice(cc * 512, (cc + 1) * 512)
                            pg = pp.tile([P, 512], f32, tag="pg", bufs=2)
                            pu = pp.tile([P, 512], f32, tag="pu", bufs=2)
                            for ht in range(HT):
                                nc.tensor.matmul(
                                    pg, wgp[:, ht, ks], xt[:, ht, cs],
                                    start=(ht == 0), stop=(ht == HT - 1))
                            for ht in range(HT):
                                nc.tensor.matmul(
                                    pu, wup[:, ht, ks], xt[:, ht, cs],
                                    start=(ht == 0), stop=(ht == HT - 1))
                            sil = sbs.tile([P, 512], f32, tag="sil")
                            nc.scalar.activation(sil, pg, AF.Silu)
                            nc.vector.tensor_mul(acts[:, i, cs], sil, pu)

            # ---- Phase 2: down GEMM (bf16), full-I psum chains -----------
            with tc.tile_pool(name="w2", bufs=2) as w2:

                def load_slab(hc):
                    hs = slice(hc * HW_, (hc + 1) * HW_)
                    wsl = w2.tile([P, IT, HW_], bf16, tag="wsl", name=f"ws{hc}")
                    for q in range(4):
                        qs = slice(q * (IT // 4), (q + 1) * (IT // 4))
                        nc.sync.dma_start(wsl[:, qs, :], wdn_v[:, qs, hs])
                    return wsl

                wsl_cur = wsl0
                wsl_next = load_slab(1)
                for hc in range(NHC):
                    hs = slice(hc * HW_, (hc + 1) * HW_)
                    for ct in range(CT):
                        ps = pp.tile([P, HW_], f32, tag="ps", bufs=2)
                        for i in range(IT):
                            nc.tensor.matmul(
                                ps,
                                acts[:, i, ct * P:(ct + 1) * P],
                                wsl_cur[:, i, :],
                                start=(i == 0), stop=(i == IT - 1))
                        osb = sbs.tile([P, HW_], f32, tag="osb")
                        nc.vector.tensor_copy(osb, ps)
                        # out-DMAs ride the scalar HWDGE queue so slab
                        # prefetches on sync are never stuck behind them
                        nc.scalar.dma_start(out_v[:, ct, hs], osb)
                    if hc + 1 < NHC:
                        wsl_cur = wsl_next
                        if hc + 2 < NHC:
                            wsl_next = load_slab(hc + 2)

    nc.compile()
    return nc


def _get_nc():
    if "nc" not in _CACHE:
        _CACHE["nc"] = _build()
    return _CACHE["nc"]


def _bf16(a):
    return np.ascontiguousarray(np.asarray(a, dtype=ml_dtypes.bfloat16))


def _run(hidden_states, w_gate_up, w_down, trace=False):
    from concourse.bass_utils import run_bass_kernel_spmd

    nc = _get_nc()
    hs = np.asarray(hidden_states, dtype=np.float32)
    in_maps = [
        {
            "xT": _bf16(hs[e].T),
            "wgu": _bf16(w_gate_up[e]),
            "wdn": _bf16(w_down[e]),
        }
        for e in range(E)
    ]
    res = run_bass_kernel_spmd(nc, in_maps, list(range(E)), trace=trace)
    output = np.stack([res.results[e]["out"] for e in range(E)], axis=0)
    return output, res


def kernel(hidden_states, w_gate_up, w_down):
    output, _ = _run(hidden_states, w_gate_up, w_down, trace=False)
    return output
